# revision 1
# baseline (speedup 1.0000x reference)
"""Trainium2 Bass kernel for nn_CorrelationLayer.

Reference computation (per sample, C=256, H=W=64, s=8):
    corr  = 0.5*(corr_branch(x0) + corr_branch(x1))        # [64, H, W]
    red   = relu(instance_norm(conv1x1(corr, w_red1)))     # b_red1 cancels in IN
    red   = conv3x3(red, w_red2) + b_red2                  # [256, H, W]
    new   = relu(conv1x1(concat(x0, red), w_adapt) + b_adapt)
    depth = instance_norm(x1)
where corr_branch(x) = l2norm_c(avgpool8(x)) . l2norm_c(x) (cosine maps).

Sharding: pure data parallel, 2 samples per core on 8 cores.

Device layout per sample: channels on partitions (2 tiles of 128), the
4096 pixels on the free dim, processed in 512-px chunks.  All matmuls
run as float32r (1 cycle/row at N>=512).  Per-pixel l2 norms come from a
replicated-ones matmul (column sum-of-squares lands broadcast across the
64 output partitions in PSUM); rsqrt is computed as Exp(-0.5*Ln(x) +
ln(0.5)) on the scalar engine (the 0.5 corr average is folded in).  The
pooled-feature l2 norm cancels the 1/64 avgpool scale, so raw block sums
are used.  Instance-norm stats use bn_stats/bn_aggr.  conv3x3 reads a
zero-padded [128, 66*66] tile through shifted access patterns.
"""

import sys

sys.path.insert(0, "/opt/trn_rl_repo")

import numpy as np
from contextlib import ExitStack

import concourse.bass as bass
import concourse.tile as tile
from concourse import bacc, mybir
from concourse.bass_utils import run_bass_kernel_spmd

AF = mybir.ActivationFunctionType
ALU = mybir.AluOpType
AX = mybir.AxisListType
F32 = mybir.dt.float32
F32R = mybir.dt.float32r

N_CORES = 8
B, C, H, W = 16, 256, 64, 64
HW = H * W            # 4096
S2 = 64               # corr_size**2
SPC = B // N_CORES    # samples per core = 2
NCH = 8               # 512-px chunks per image
CHUNK = HW // NCH     # 512
EPS_IN = 1e-5
LN_HALF = float(np.log(0.5))

_CACHE = {}


def _r(ap):
    return ap.bitcast(F32R)


class _Bacc(bacc.Bacc):
    """Bacc whose ACT-table chooser is pinned to the one set that covers
    every function this kernel uses (square/ln/exp/copy/identity/relu).
    The default greedy chooser alternates between two partial sets and
    inserts ~57 per-chunk table loads at 1283 ns each."""

    _ACT_SET = "natural_log_exp_and_others"

    def insert_act_table_loads(self):
        import concourse.mybir as _mb
        from concourse.hw_specs import get_activation_tables
        import bass_rust as _br
        has_activation = any(
            isinstance(i, _mb.InstActivation)
            for blk in self.main_func.blocks
            for i in blk.instructions
        )
        if not has_activation:
            return
        tables = [
            (name, funcs if name == self._ACT_SET else set())
            for name, funcs in get_activation_tables(self.m.arch).items()
        ]
        _br.insert_act_table_loads(self, tables)


def _build_program():
    nc = _Bacc("TRN2", target_bir_lowering=False, debug=False,
               num_devices=N_CORES)

    x0_d = nc.dram_tensor("x0", [SPC, C, HW], F32, kind="ExternalInput").ap()
    x1_d = nc.dram_tensor("x1", [SPC, C, HW], F32, kind="ExternalInput").ap()
    w1t_d = nc.dram_tensor("w1t", [S2, C], F32, kind="ExternalInput").ap()
    w2t_d = nc.dram_tensor("w2t", [2, 128, 9 * C], F32, kind="ExternalInput").ap()
    wat_d = nc.dram_tensor("wat", [4, 128, C], F32, kind="ExternalInput").ap()
    b2_d = nc.dram_tensor("b2", [2, 128], F32, kind="ExternalInput").ap()
    ba_d = nc.dram_tensor("ba", [2, 128], F32, kind="ExternalInput").ap()
    nf_d = nc.dram_tensor("nf", [SPC, C, HW], F32, kind="ExternalOutput").ap()
    df_d = nc.dram_tensor("df", [SPC, C, HW], F32, kind="ExternalOutput").ap()

    with tile.TileContext(nc) as tc, ExitStack() as ctx:
        tcp = lambda **kw: ctx.enter_context(tc.tile_pool(**kw))
        p_w = tcp(name="weights", bufs=1)
        p_stream = tcp(name="stream", bufs=8)         # [128,512] x-chunk loads
        p_sq = tcp(name="sq", bufs=3)                 # [128,512] squares
        p_p1 = tcp(name="p1", bufs=8)                 # [128,512] pool stage1
        p_ipx = tcp(name="ipx", bufs=9)               # [128,512] 0.5/pixnorm
        p_small = tcp(name="small", bufs=40)          # stats & pooled tiles
        p_corr = tcp(name="corr", bufs=1)             # [64,4096]
        p_big = tcp(name="big", bufs=2)               # [128,4096] RR then R2
        p_pad = tcp(name="pad", bufs=2)               # [128,4356]
        p_tt = tcp(name="tt", bufs=4)                 # [64,512] combine tmp
        p_df = tcp(name="df", bufs=4)                 # [128,512] depth out
        p_nf = tcp(name="nf", bufs=2)                 # [128,1024] new_feat out
        p_xa = tcp(name="xa", bufs=2)                 # [128,1024] x0 reload

        # PSUM: 8 banks statically split per phase so no cross-phase
        # starvation cycle is possible.
        ps_i = tcp(name="ps_i", bufs=2, space="PSUM")    # phase I cs/nk
        ps_ii = tcp(name="ps_ii", bufs=3, space="PSUM")  # phase II corr/red1
        ps_iii = tcp(name="ps_iii", bufs=3, space="PSUM")  # conv3x3/adapt

        # ---- constants & weights (once) ----
        ones_f = p_w.tile([128, 128], F32)
        nc.vector.memset(ones_f[:], 1.0)
        ones = p_w.tile([128, 128], F32R)
        nc.scalar.copy(ones[:], ones_f[:])
        eps_c = p_w.tile([128, 1], F32)
        nc.vector.memset(eps_c[:], EPS_IN)
        lnh_c = p_w.tile([128, 1], F32)
        nc.vector.memset(lnh_c[:], LN_HALF)
        w1t_sb = p_w.tile([S2, C], F32R)
        nc.sync.dma_start(w1t_sb[:], _r(w1t_d[:]))
        w2t_sb = []
        for kt in range(2):
            w2 = p_w.tile([128, 9 * C], F32R, name=f"w2t_sb{kt}")
            nc.sync.dma_start(w2[:], _r(w2t_d[kt]))
            w2t_sb.append(w2)
        wat_sb = p_w.tile([128, 4 * C], F32R)
        for kt in range(4):
            nc.sync.dma_start(wat_sb[:, kt * C:(kt + 1) * C], _r(wat_d[kt]))
        b2_sb = p_w.tile([128, 2], F32)
        ba_sb = p_w.tile([128, 2], F32)
        for mt in range(2):
            nc.sync.dma_start(b2_sb[:, mt:mt + 1], b2_d[mt].unsqueeze(1))
            nc.sync.dma_start(ba_sb[:, mt:mt + 1], ba_d[mt].unsqueeze(1))

        st = [dict() for _ in range(SPC)]

        def phase1_head(s):
            d = st[s]
            # ===== phase I: pooled sums, x1 IN stats, pixel norms =====
            # 0.5/pixel_l2norm for both inputs, packed [input0: rows 0-63,
            # input1: rows 64-127] per 512-px chunk.
            d['ipx'] = [p_ipx.tile([128, CHUNK], F32, name=f"ipx_{s}_{ch}",
                                   tag="ipx") for ch in range(NCH)]
            d['p1t'] = [[p_p1.tile([128, CHUNK], F32, name=f"p1_{s}_{i}_{t}",
                                   tag="p1") for t in range(2)]
                        for i in range(2)]
            d['sscols'] = [p_small.tile([128, NCH], F32, name=f"ss1_{s}_{t}",
                                        tag="small") for t in range(2)]

        def phase1_chunk(s, ch):
            d = st[s]
            ipx, p1t, sscols = d['ipx'], d['p1t'], d['sscols']
            csp = [ps_i.tile([S2, CHUNK], F32, name=f"cs{i}_{s}_{ch}",
                             tag="ps_i") for i in range(2)]
            for i, xd in ((0, x0_d), (1, x1_d)):
                for t in range(2):
                    xc = p_stream.tile([128, CHUNK], F32,
                                       name=f"xi_{s}_{i}_{t}_{ch}",
                                       tag="stream")
                    nc.sync.dma_start(
                        xc[:], xd[s, t * 128:(t + 1) * 128,
                                  ch * CHUNK:(ch + 1) * CHUNK])
                    # pooled stage 1: sum over w within groups of 8
                    nc.vector.tensor_reduce(
                        p1t[i][t][:, ch * 64:(ch + 1) * 64],
                        xc[:].rearrange("p (g w) -> p g w", w=8),
                        AX.X, ALU.add)
                    sqc = p_sq.tile([128, CHUNK], F32R,
                                    name=f"sq_{s}_{i}_{t}_{ch}", tag="sq")
                    if i == 0:
                        nc.scalar.square(sqc[:], xc[:])
                    else:
                        # square + channel sum-of-squares in one DVE op
                        nc.vector.scalar_tensor_tensor(
                            sqc[:], xc[:], 1.0, xc[:], ALU.mult, ALU.mult,
                            accum_out=sscols[t][:, ch:ch + 1])
                    nc.tensor.matmul(csp[i][:],
                                     _r(ones[:, :S2]), _r(sqc[:]),
                                     start=(t == 0), stop=(t == 1))
            for i in range(2):
                half = ipx[ch][i * S2:(i + 1) * S2, :]
                nc.scalar.activation(half, csp[i][:], AF.Ln)
                nc.scalar.activation(half, half, AF.Exp,
                                     bias=lnh_c[0:S2, :], scale=-0.5)

        def phase1_tail(s):
            d = st[s]
            p1t, sscols = d['p1t'], d['sscols']
            # pooled stage 2 + l2 norm of pooled features
            khat = []
            for i in range(2):
                kh_t = []
                for t in range(2):
                    pk = p_small.tile([128, S2], F32,
                                      name=f"pooled_{s}_{i}_{t}", tag="small")
                    # p1 index = 64*a + 8*r + w  ->  sum over r
                    nc.vector.tensor_reduce(
                        pk[:],
                        p1t[i][t][:].rearrange("p (a r w) -> p a w r",
                                               a=8, r=8),
                        AX.X, ALU.add)
                    kh_t.append(pk)
                nkp = ps_i.tile([128, S2], F32, name=f"nk_{s}_{i}", tag="ps_i")
                psq = []
                for t in range(2):
                    pq = p_small.tile([128, S2], F32R, name=f"psq_{s}_{i}_{t}",
                                      tag="small")
                    nc.vector.tensor_tensor(pq[:], kh_t[t][:], kh_t[t][:],
                                            ALU.mult)
                    psq.append(pq)
                for t in range(2):
                    nc.tensor.matmul(nkp[:], _r(ones[:]), _r(psq[t][:]),
                                     start=(t == 0), stop=(t == 1))
                nk_sb = p_small.tile([128, S2], F32, name=f"nk_sb_{s}_{i}",
                                     tag="small")
                nc.scalar.activation(nk_sb[:], nkp[:], AF.Ln)
                invk = p_small.tile([128, S2], F32, name=f"invk_{s}_{i}",
                                    tag="small")
                nc.scalar.activation(invk[:], nk_sb[:], AF.Exp, scale=-0.5)
                kh = []
                for t in range(2):
                    k2 = p_small.tile([128, S2], F32R, name=f"khat_{s}_{i}_{t}",
                                      tag="small")
                    nc.vector.tensor_tensor(k2[:], kh_t[t][:], invk[:],
                                            ALU.mult)
                    kh.append(k2)
                khat.append(kh)
        
            # depth-feat (instance norm of x1): var = E[x^2] - E[x]^2
            istd1, bneg1 = [], []
            for t in range(2):
                mv = p_small.tile([128, 2], F32, name=f"mv1_{s}_{t}",
                                  tag="small")
                nc.vector.tensor_reduce(mv[:, 0:1], sscols[t][:], AX.X,
                                        ALU.add)
                nc.vector.tensor_reduce(mv[:, 1:2], p1t[1][t][:], AX.X,
                                        ALU.add)
                mvn = p_small.tile([128, 2], F32, name=f"mvn_{s}_{t}",
                                   tag="small")
                nc.vector.tensor_scalar(mvn[:], mv[:], 1.0 / HW, None,
                                        ALU.mult)
                msq = p_small.tile([128, 1], F32, name=f"msq_{s}_{t}",
                                   tag="small")
                nc.vector.tensor_tensor(msq[:], mvn[:, 1:2], mvn[:, 1:2],
                                        ALU.mult)
                var = p_small.tile([128, 1], F32, name=f"var1_{s}_{t}",
                                   tag="small")
                nc.vector.tensor_tensor(var[:], mvn[:, 0:1], msq[:],
                                        ALU.subtract)
                std = p_small.tile([128, 1], F32, name=f"std1_{s}_{t}",
                                   tag="small")
                nc.scalar.activation(std[:], var[:], AF.Ln, bias=eps_c[:])
                ist = p_small.tile([128, 1], F32, name=f"istd1_{s}_{t}",
                                   tag="small")
                nc.scalar.activation(ist[:], std[:], AF.Exp, scale=-0.5)
                bn = p_small.tile([128, 1], F32, name=f"bneg1_{s}_{t}",
                                  tag="small")
                nc.vector.scalar_tensor_tensor(bn[:], mvn[:, 1:2], -1.0,
                                               ist[:], ALU.mult, ALU.mult)
                istd1.append(ist)
                bneg1.append(bn)
        
            d['khat'] = khat
            d['istd1'] = istd1; d['bneg1'] = bneg1

        def phase2_head(s):
            d = st[s]
            # ===== phase II: corr, red1, IN(red) stats, depth out =====
            d['corr'] = p_corr.tile([S2, HW], F32R, name=f"corr_{s}",
                                    tag="corr")
            d['rr'] = [p_big.tile([128, HW], F32, name=f"rr_{s}_{mt}",
                                  tag="big") for mt in range(2)]
            d['bnsr'] = [p_small.tile([128, NCH * 6], F32,
                                      name=f"bnsr_{s}_{mt}", tag="small")
                         for mt in range(2)]

        def phase2_chunk(s, ch):
            d = st[s]
            ipx = d['ipx']; khat = d['khat']
            istd1 = d['istd1']; bneg1 = d['bneg1']
            corr_sb = d['corr']; rr = d['rr']; bnsr = d['bnsr']
            if True:
                cp = [ps_ii.tile([S2, CHUNK], F32, name=f"c{i}_{s}_{ch}",
                                tag="ps_ii") for i in range(2)]
                for i, xd in ((0, x0_d), (1, x1_d)):
                    for t in range(2):
                        xc = p_stream.tile([128, CHUNK], F32R,
                                           name=f"xii{i}_{s}_{t}_{ch}",
                                           tag="stream")
                        nc.sync.dma_start(
                            xc[:], _r(xd[s, t * 128:(t + 1) * 128,
                                        ch * CHUNK:(ch + 1) * CHUNK]))
                        nc.tensor.matmul(cp[i][:],
                                         _r(khat[i][t][:]), _r(xc[:]),
                                         start=(t == 0), stop=(t == 1))
                        if i == 1:
                            # depth_feat chunk (reuses the x1 load)
                            dfc = p_df.tile([128, CHUNK], F32,
                                            name=f"dfc_{s}_{t}_{ch}", tag="df")
                            nc.vector.tensor_scalar(dfc[:],
                                                    xc[:].bitcast(F32),
                                                    istd1[t][:], bneg1[t][:],
                                                    ALU.mult, ALU.add)
                            nc.gpsimd.dma_start(
                                df_d[s, t * 128:(t + 1) * 128,
                                     ch * CHUNK:(ch + 1) * CHUNK], dfc[:])
                sl = slice(ch * CHUNK, (ch + 1) * CHUNK)
                tts = []
                for i in range(2):
                    t_ = p_tt.tile([S2, CHUNK], F32, name=f"tt{i}_{s}_{ch}",
                                   tag="tt")
                    nc.vector.tensor_tensor(t_[:], cp[i][:],
                                            ipx[ch][i * S2:(i + 1) * S2, :],
                                            ALU.mult)
                    tts.append(t_)
                nc.vector.tensor_tensor(corr_sb[:, sl], tts[0][:], tts[1][:],
                                        ALU.add)
                # red1 = w1 @ corr  (K = 64)
                for mt in range(2):
                    rrp = ps_ii.tile([128, CHUNK], F32,
                                    name=f"rrp_{s}_{ch}_{mt}", tag="ps_ii")
                    nc.tensor.matmul(rrp[:],
                                     _r(w1t_sb[:, mt * 128:(mt + 1) * 128]),
                                     _r(corr_sb[:, sl]), start=True, stop=True)
                    nc.scalar.copy(rr[mt][:, sl], rrp[:])
                    nc.vector.bn_stats(bnsr[mt][:, ch * 6:(ch + 1) * 6],
                                       rrp[:])

        def phase2_tail(s):
            d = st[s]
            rr = d['rr']; bnsr = d['bnsr']
            # IN(red) scale/bias, then relu into padded conv input
            red_pad = []
            for mt in range(2):
                mv = p_small.tile([128, 2], F32, name=f"mvr_{s}_{mt}",
                                  tag="small")
                nc.vector.bn_aggr(mv[:], bnsr[mt][:])
                std = p_small.tile([128, 1], F32, name=f"stdr_{s}_{mt}",
                                   tag="small")
                nc.scalar.activation(std[:], mv[:, 1:2], AF.Ln, bias=eps_c[:])
                ist = p_small.tile([128, 1], F32, name=f"istdr_{s}_{mt}",
                                   tag="small")
                nc.scalar.activation(ist[:], std[:], AF.Exp, scale=-0.5)
                bn = p_small.tile([128, 1], F32, name=f"bnegr_{s}_{mt}",
                                  tag="small")
                nc.vector.scalar_tensor_tensor(bn[:], mv[:, 0:1], -1.0, ist[:],
                                               ALU.mult, ALU.mult)
                pad = p_pad.tile([128, 66 * 66], F32R, name=f"pad_{s}_{mt}",
                                 tag="pad")
                pv = pad[:].rearrange("p (h w) -> p h w", w=66)
                for brd in (pv[:, 0:1, :], pv[:, 65:66, :],
                            pv[:, 1:65, 0:1], pv[:, 1:65, 65:66]):
                    nc.scalar.activation(brd, brd.bitcast(F32), AF.Copy,
                                         scale=0.0)
                nc.scalar.activation(
                    pv[:, 1:65, 1:65],
                    rr[mt][:].rearrange("p (h w) -> p h w", w=64),
                    AF.Relu, bias=bn[:], scale=ist[:])
                red_pad.append(pad)
            d['red_pad'] = red_pad

        def phase3(s, weave=None):
            d = st[s]
            red_pad = d['red_pad']
            # ===== phase III: conv3x3 ===== (r2 reuses the rr slots)
            r2 = [p_big.tile([128, HW], F32R, name=f"r2_{s}_{mt}", tag="big")
                  for mt in range(2)]
            d['r2'] = r2
            pvs = [red_pad[kt][:].rearrange("p (h w) -> p h w", w=66)
                   for kt in range(2)]
            for ch in range(NCH):               # 512-px chunks (8 rows)
                if weave is not None:
                    weave(ch)
                for mt in range(2):
                    c3p = ps_iii.tile([128, CHUNK], F32,
                                      name=f"c3_{s}_{mt}_{ch}", tag="ps_iii")
                    y0 = ch * 8
                    first = True
                    for off in range(9):
                        dy, dx = off // 3, off % 3
                        for kt in range(2):
                            lhs = w2t_sb[kt][:, off * C + mt * 128:
                                             off * C + mt * 128 + 128]
                            rhs = pvs[kt][:, y0 + dy:y0 + dy + 8, dx:dx + 64]
                            nc.tensor.matmul(
                                c3p[:], _r(lhs), _r(rhs),
                                start=first, stop=(off == 8 and kt == 1))
                            first = False
                    nc.scalar.activation(
                        r2[mt][:, ch * CHUNK:(ch + 1) * CHUNK],
                        c3p[:], AF.Identity, bias=b2_sb[:, mt:mt + 1])
                if weave is not None:
                    weave(NCH + ch)

        def phase4_unit(s, mt, g):
            d = st[s]
            r2 = d['r2']
            # ===== phase IV: adapt conv1x1 + relu (one 1024-px group) =====
            if True:
                if True:
                    gsl = slice(g * 1024, (g + 1) * 1024)
                    xas = []
                    for kt in range(2):
                        xa = p_xa.tile([128, 1024], F32R,
                                       name=f"xa_{s}_{mt}_{g}_{kt}", tag="xa")
                        nc.sync.dma_start(
                            xa[:], _r(x0_d[s, kt * 128:(kt + 1) * 128, gsl]))
                        xas.append(xa)
                    for cc in range(2):
                        ap_ = ps_ii.tile([128, CHUNK], F32,
                                        name=f"aps_{s}_{mt}_{g}_{cc}",
                                        tag="ps_ii")
                        csl = slice(cc * 512, (cc + 1) * 512)
                        for kt in range(4):
                            lhs = wat_sb[:, kt * C + mt * 128:
                                         kt * C + mt * 128 + 128]
                            if kt < 2:
                                rhs = xas[kt][:, csl]
                            else:
                                rhs = r2[kt - 2][:, g * 1024 + cc * 512:
                                                 g * 1024 + (cc + 1) * 512]
                            nc.tensor.matmul(ap_[:], _r(lhs), _r(rhs),
                                             start=(kt == 0), stop=(kt == 3))
                        nfc = p_nf.tile([128, CHUNK], F32,
                                        name=f"nf_{s}_{mt}_{g}_{cc}", tag="nf")
                        nc.scalar.activation(nfc[:], ap_[:], AF.Relu,
                                             bias=ba_sb[:, mt:mt + 1])
                        nc.gpsimd.dma_start(
                            nf_d[s, mt * 128:(mt + 1) * 128,
                                 g * 1024 + cc * 512:g * 1024 + (cc + 1) * 512],
                            nfc[:])
        
        
        phase1_head(0)
        for ch in range(NCH):
            phase1_chunk(0, ch)
        phase1_tail(0)
        phase2_head(0)
        for ch in range(NCH):
            phase2_chunk(0, ch)
        phase2_tail(0)
        phase1_head(1)

        # Weave slots fire in order 0,8,1,9,2,10,3,11,4,12,5,13,6,14,7,15
        # (slot k fires before conv chunk k; slot 8+k right after chunk k).
        # Pack sample 1's phase-I chunks into the first 8 fired slots so its
        # tail (and then phase II) unblocks by mid-window; adapt groups fill
        # the second half once their r2 chunks exist.
        _w0 = {0: 0, 8: 1, 1: 2, 9: 3, 2: 4, 10: 5, 3: 6, 11: 7}

        def _weave0(k):
            if k in _w0:
                phase1_chunk(1, _w0[k])
            elif k == 4:
                phase1_tail(1)
            elif k == 12:
                phase4_unit(0, 0, 0)
                phase4_unit(0, 1, 0)
            elif k == 5:
                phase4_unit(0, 0, 1)
                phase4_unit(0, 1, 1)
            elif k == 13:
                phase4_unit(0, 0, 2)
                phase4_unit(0, 1, 2)

        phase3(0, weave=_weave0)
        phase4_unit(0, 0, 3)
        phase4_unit(0, 1, 3)
        phase2_head(1)
        for ch in range(NCH):
            phase2_chunk(1, ch)
        phase2_tail(1)

        def _weave1(k):
            # weave sample 1's adapt groups into the conv3x3(1) window
            if k in (NCH + 1, NCH + 3, NCH + 5):
                g = (k - NCH - 1) // 2
                phase4_unit(1, 0, g)
                phase4_unit(1, 1, g)

        phase3(1, weave=_weave1)
        phase4_unit(1, 0, 3)
        phase4_unit(1, 1, 3)

    nc.compile()
    return nc


def _get_program():
    if "nc" not in _CACHE:
        _CACHE["nc"] = _build_program()
    return _CACHE["nc"]


def _prep_weights(w_red1, w_red2, w_adapt, b_red2, b_adapt):
    w1t = np.ascontiguousarray(w_red1[:, :, 0, 0].T)                  # [64,256]
    w2 = w_red2.transpose(2, 3, 1, 0).reshape(9, C, C)                # off,ci,co
    w2t = np.ascontiguousarray(
        w2.reshape(9, 2, 128, C).transpose(1, 2, 0, 3).reshape(2, 128, 9 * C))
    wat = np.ascontiguousarray(w_adapt[:, :, 0, 0].T.reshape(4, 128, C))
    b2 = np.ascontiguousarray(b_red2.reshape(2, 128))
    ba = np.ascontiguousarray(b_adapt.reshape(2, 128))
    return w1t, w2t, wat, b2, ba


def make_in_maps(x0, x1, w_red1, b_red1, w_red2, b_red2, w_adapt, b_adapt):
    w1t, w2t, wat, b2, ba = _prep_weights(
        np.asarray(w_red1, np.float32), np.asarray(w_red2, np.float32),
        np.asarray(w_adapt, np.float32), np.asarray(b_red2, np.float32),
        np.asarray(b_adapt, np.float32))
    x0 = np.asarray(x0, np.float32).reshape(B, C, HW)
    x1 = np.asarray(x1, np.float32).reshape(B, C, HW)
    in_maps = []
    for i in range(N_CORES):
        sl = slice(i * SPC, (i + 1) * SPC)
        in_maps.append({
            "x0": np.ascontiguousarray(x0[sl]),
            "x1": np.ascontiguousarray(x1[sl]),
            "w1t": w1t, "w2t": w2t, "wat": wat, "b2": b2, "ba": ba,
        })
    return in_maps


def kernel(x0, x1, w_red1, b_red1, w_red2, b_red2, w_adapt, b_adapt):
    nc = _get_program()
    in_maps = make_in_maps(x0, x1, w_red1, b_red1, w_red2, b_red2,
                           w_adapt, b_adapt)
    res = run_bass_kernel_spmd(nc, in_maps, list(range(N_CORES)))
    nf = np.concatenate([res.results[i]["nf"] for i in range(N_CORES)], axis=0)
    df = np.concatenate([res.results[i]["df"] for i in range(N_CORES)], axis=0)
    return (nf.reshape(B, C, H, W).astype(np.float32),
            df.reshape(B, C, H, W).astype(np.float32))



# revision 2
# speedup vs baseline: 1.0723x; 1.0723x over previous
"""Trainium2 Bass kernel for nn_CorrelationLayer.

Reference computation (per sample, C=256, H=W=64, s=8):
    corr  = 0.5*(corr_branch(x0) + corr_branch(x1))        # [64, H, W]
    red   = relu(instance_norm(conv1x1(corr, w_red1)))     # b_red1 cancels in IN
    red   = conv3x3(red, w_red2) + b_red2                  # [256, H, W]
    new   = relu(conv1x1(concat(x0, red), w_adapt) + b_adapt)
    depth = instance_norm(x1)
where corr_branch(x) = l2norm_c(avgpool8(x)) . l2norm_c(x) (cosine maps).

Sharding: pure data parallel, 2 samples per core on 8 cores.

Device layout per sample: channels on partitions (2 tiles of 128), the
4096 pixels on the free dim, processed in 512-px chunks.  All matmul
operands are bf16 (1 cycle/row on the PE array vs 4 for fp32), with fp32
PSUM accumulation; stats/normalization math stays fp32.  Inputs and
weights are converted to bf16 on the host, halving HBM traffic.
Per-pixel l2 norms come from a replicated-ones matmul (column
sum-of-squares lands broadcast across the 64 output partitions in PSUM);
rsqrt is computed as Exp(-0.5*Ln(x) + ln(0.5)) on the scalar engine (the
0.5 corr average is folded in).  The pooled-feature l2 norm cancels the
1/64 avgpool scale, so raw block sums are used.  Instance-norm stats use
bn_stats/bn_aggr.  conv3x3 reads a zero-padded [128, 66*66] tile through
shifted access patterns.
"""

import sys

sys.path.insert(0, "/opt/trn_rl_repo")

import numpy as np
import ml_dtypes
from contextlib import ExitStack

import concourse.bass as bass
import concourse.tile as tile
from concourse import bacc, mybir
from concourse.bass_utils import run_bass_kernel_spmd

AF = mybir.ActivationFunctionType
ALU = mybir.AluOpType
AX = mybir.AxisListType
F32 = mybir.dt.float32
BF16 = mybir.dt.bfloat16
BF16_NP = ml_dtypes.bfloat16

N_CORES = 8
B, C, H, W = 16, 256, 64, 64
HW = H * W            # 4096
S2 = 64               # corr_size**2
SPC = B // N_CORES    # samples per core = 2
NCH = 8               # 512-px chunks per image
CHUNK = HW // NCH     # 512
EPS_IN = 1e-5
LN_HALF = float(np.log(0.5))

_CACHE = {}


class _Bacc(bacc.Bacc):
    """Bacc whose ACT-table chooser is pinned to the one set that covers
    every function this kernel uses (square/ln/exp/copy/identity/relu).
    The default greedy chooser alternates between two partial sets and
    inserts ~57 per-chunk table loads at 1283 ns each."""

    _ACT_SET = "natural_log_exp_and_others"

    def insert_act_table_loads(self):
        import concourse.mybir as _mb
        from concourse.hw_specs import get_activation_tables
        import bass_rust as _br
        has_activation = any(
            isinstance(i, _mb.InstActivation)
            for blk in self.main_func.blocks
            for i in blk.instructions
        )
        if not has_activation:
            return
        tables = [
            (name, funcs if name == self._ACT_SET else set())
            for name, funcs in get_activation_tables(self.m.arch).items()
        ]
        _br.insert_act_table_loads(self, tables)


def _build_program():
    nc = _Bacc("TRN2", target_bir_lowering=False, debug=False,
               num_devices=N_CORES)

    x0_d = nc.dram_tensor("x0", [SPC, C, HW], BF16, kind="ExternalInput").ap()
    x1_d = nc.dram_tensor("x1", [SPC, C, HW], BF16, kind="ExternalInput").ap()
    w1t_d = nc.dram_tensor("w1t", [S2, C], BF16, kind="ExternalInput").ap()
    w2t_d = nc.dram_tensor("w2t", [2, 128, 9 * C], BF16, kind="ExternalInput").ap()
    wat_d = nc.dram_tensor("wat", [4, 128, C], BF16, kind="ExternalInput").ap()
    b2_d = nc.dram_tensor("b2", [2, 128], F32, kind="ExternalInput").ap()
    ba_d = nc.dram_tensor("ba", [2, 128], F32, kind="ExternalInput").ap()
    nf_d = nc.dram_tensor("nf", [SPC, C, HW], F32, kind="ExternalOutput").ap()
    df_d = nc.dram_tensor("df", [SPC, C, HW], F32, kind="ExternalOutput").ap()

    with tile.TileContext(nc) as tc, ExitStack() as ctx:
        tcp = lambda **kw: ctx.enter_context(tc.tile_pool(**kw))
        p_w = tcp(name="weights", bufs=1)
        p_stream = tcp(name="stream", bufs=8)         # [128,512] x-chunk loads
        p_sq = tcp(name="sq", bufs=3)                 # [128,512] squares
        p_p1 = tcp(name="p1", bufs=8)                 # [128,512] pool stage1
        p_ipx = tcp(name="ipx", bufs=9)               # [128,512] 0.5/pixnorm
        p_small = tcp(name="small", bufs=40)          # stats & pooled tiles
        p_corr = tcp(name="corr", bufs=1)             # [64,4096]
        p_big = tcp(name="big", bufs=2)               # [128,4096] RR then R2
        p_pad = tcp(name="pad", bufs=2)               # [128,4356]
        p_tt = tcp(name="tt", bufs=4)                 # [64,512] combine tmp
        p_df = tcp(name="df", bufs=4)                 # [128,512] depth out
        p_nf = tcp(name="nf", bufs=2)                 # [128,1024] new_feat out
        p_xa = tcp(name="xa", bufs=2)                 # [128,1024] x0 reload

        # PSUM: 8 banks statically split per phase so no cross-phase
        # starvation cycle is possible.
        ps_i = tcp(name="ps_i", bufs=2, space="PSUM")    # phase I cs/nk
        ps_ii = tcp(name="ps_ii", bufs=3, space="PSUM")  # phase II corr/red1
        ps_iii = tcp(name="ps_iii", bufs=3, space="PSUM")  # conv3x3/adapt

        # ---- constants & weights (once) ----
        ones_f = p_w.tile([128, 128], F32)
        nc.vector.memset(ones_f[:], 1.0)
        ones = p_w.tile([128, 128], BF16)
        nc.scalar.copy(ones[:], ones_f[:])
        eps_c = p_w.tile([128, 1], F32)
        nc.vector.memset(eps_c[:], EPS_IN)
        lnh_c = p_w.tile([128, 1], F32)
        nc.vector.memset(lnh_c[:], LN_HALF)
        w1t_sb = p_w.tile([S2, C], BF16)
        nc.sync.dma_start(w1t_sb[:], w1t_d[:])
        w2t_sb = []
        for kt in range(2):
            w2 = p_w.tile([128, 9 * C], BF16, name=f"w2t_sb{kt}")
            nc.sync.dma_start(w2[:], w2t_d[kt])
            w2t_sb.append(w2)
        wat_sb = p_w.tile([128, 4 * C], BF16)
        for kt in range(4):
            nc.sync.dma_start(wat_sb[:, kt * C:(kt + 1) * C], wat_d[kt])
        b2_sb = p_w.tile([128, 2], F32)
        ba_sb = p_w.tile([128, 2], F32)
        for mt in range(2):
            nc.sync.dma_start(b2_sb[:, mt:mt + 1], b2_d[mt].unsqueeze(1))
            nc.sync.dma_start(ba_sb[:, mt:mt + 1], ba_d[mt].unsqueeze(1))

        st = [dict() for _ in range(SPC)]

        def phase1_head(s):
            d = st[s]
            # ===== phase I: pooled sums, x1 IN stats, pixel norms =====
            # 0.5/pixel_l2norm for both inputs, packed [input0: rows 0-63,
            # input1: rows 64-127] per 512-px chunk.
            d['ipx'] = [p_ipx.tile([128, CHUNK], F32, name=f"ipx_{s}_{ch}",
                                   tag="ipx") for ch in range(NCH)]
            d['p1t'] = [[p_p1.tile([128, CHUNK], F32, name=f"p1_{s}_{i}_{t}",
                                   tag="p1") for t in range(2)]
                        for i in range(2)]
            d['sscols'] = [p_small.tile([128, NCH], F32, name=f"ss1_{s}_{t}",
                                        tag="small") for t in range(2)]

        def phase1_chunk(s, ch):
            d = st[s]
            ipx, p1t, sscols = d['ipx'], d['p1t'], d['sscols']
            csp = [ps_i.tile([S2, CHUNK], F32, name=f"cs{i}_{s}_{ch}",
                             tag="ps_i") for i in range(2)]
            for i, xd in ((0, x0_d), (1, x1_d)):
                for t in range(2):
                    xc = p_stream.tile([128, CHUNK], BF16,
                                       name=f"xi_{s}_{i}_{t}_{ch}",
                                       tag="stream")
                    nc.sync.dma_start(
                        xc[:], xd[s, t * 128:(t + 1) * 128,
                                  ch * CHUNK:(ch + 1) * CHUNK])
                    # pooled stage 1: sum over w within groups of 8
                    nc.vector.tensor_reduce(
                        p1t[i][t][:, ch * 64:(ch + 1) * 64],
                        xc[:].rearrange("p (g w) -> p g w", w=8),
                        AX.X, ALU.add)
                    sqc = p_sq.tile([128, CHUNK], BF16,
                                    name=f"sq_{s}_{i}_{t}_{ch}", tag="sq")
                    if i == 0:
                        nc.scalar.square(sqc[:], xc[:])
                    else:
                        # square + channel sum-of-squares in one DVE op
                        nc.vector.scalar_tensor_tensor(
                            sqc[:], xc[:], 1.0, xc[:], ALU.mult, ALU.mult,
                            accum_out=sscols[t][:, ch:ch + 1])
                    nc.tensor.matmul(csp[i][:],
                                     ones[:, :S2], sqc[:],
                                     start=(t == 0), stop=(t == 1))
            for i in range(2):
                half = ipx[ch][i * S2:(i + 1) * S2, :]
                nc.scalar.activation(half, csp[i][:], AF.Ln)
                nc.scalar.activation(half, half, AF.Exp,
                                     bias=lnh_c[0:S2, :], scale=-0.5)

        def phase1_tail(s):
            d = st[s]
            p1t, sscols = d['p1t'], d['sscols']
            # pooled stage 2 + l2 norm of pooled features
            khat = []
            for i in range(2):
                kh_t = []
                for t in range(2):
                    pk = p_small.tile([128, S2], F32,
                                      name=f"pooled_{s}_{i}_{t}", tag="small")
                    # p1 index = 64*a + 8*r + w  ->  sum over r
                    nc.vector.tensor_reduce(
                        pk[:],
                        p1t[i][t][:].rearrange("p (a r w) -> p a w r",
                                               a=8, r=8),
                        AX.X, ALU.add)
                    kh_t.append(pk)
                nkp = ps_i.tile([128, S2], F32, name=f"nk_{s}_{i}", tag="ps_i")
                psq = []
                for t in range(2):
                    pq = p_small.tile([128, S2], BF16, name=f"psq_{s}_{i}_{t}",
                                      tag="small")
                    nc.vector.tensor_tensor(pq[:], kh_t[t][:], kh_t[t][:],
                                            ALU.mult)
                    psq.append(pq)
                for t in range(2):
                    nc.tensor.matmul(nkp[:], ones[:], psq[t][:],
                                     start=(t == 0), stop=(t == 1))
                nk_sb = p_small.tile([128, S2], F32, name=f"nk_sb_{s}_{i}",
                                     tag="small")
                nc.scalar.activation(nk_sb[:], nkp[:], AF.Ln)
                invk = p_small.tile([128, S2], F32, name=f"invk_{s}_{i}",
                                    tag="small")
                nc.scalar.activation(invk[:], nk_sb[:], AF.Exp, scale=-0.5)
                kh = []
                for t in range(2):
                    k2 = p_small.tile([128, S2], BF16, name=f"khat_{s}_{i}_{t}",
                                      tag="small")
                    nc.vector.tensor_tensor(k2[:], kh_t[t][:], invk[:],
                                            ALU.mult)
                    kh.append(k2)
                khat.append(kh)

            # depth-feat (instance norm of x1): var = E[x^2] - E[x]^2
            istd1, bneg1 = [], []
            for t in range(2):
                mv = p_small.tile([128, 2], F32, name=f"mv1_{s}_{t}",
                                  tag="small")
                nc.vector.tensor_reduce(mv[:, 0:1], sscols[t][:], AX.X,
                                        ALU.add)
                nc.vector.tensor_reduce(mv[:, 1:2], p1t[1][t][:], AX.X,
                                        ALU.add)
                mvn = p_small.tile([128, 2], F32, name=f"mvn_{s}_{t}",
                                   tag="small")
                nc.vector.tensor_scalar(mvn[:], mv[:], 1.0 / HW, None,
                                        ALU.mult)
                msq = p_small.tile([128, 1], F32, name=f"msq_{s}_{t}",
                                   tag="small")
                nc.vector.tensor_tensor(msq[:], mvn[:, 1:2], mvn[:, 1:2],
                                        ALU.mult)
                var = p_small.tile([128, 1], F32, name=f"var1_{s}_{t}",
                                   tag="small")
                nc.vector.tensor_tensor(var[:], mvn[:, 0:1], msq[:],
                                        ALU.subtract)
                std = p_small.tile([128, 1], F32, name=f"std1_{s}_{t}",
                                   tag="small")
                nc.scalar.activation(std[:], var[:], AF.Ln, bias=eps_c[:])
                ist = p_small.tile([128, 1], F32, name=f"istd1_{s}_{t}",
                                   tag="small")
                nc.scalar.activation(ist[:], std[:], AF.Exp, scale=-0.5)
                bn = p_small.tile([128, 1], F32, name=f"bneg1_{s}_{t}",
                                  tag="small")
                nc.vector.scalar_tensor_tensor(bn[:], mvn[:, 1:2], -1.0,
                                               ist[:], ALU.mult, ALU.mult)
                istd1.append(ist)
                bneg1.append(bn)

            d['khat'] = khat
            d['istd1'] = istd1; d['bneg1'] = bneg1

        def phase2_head(s):
            d = st[s]
            # ===== phase II: corr, red1, IN(red) stats, depth out =====
            d['corr'] = p_corr.tile([S2, HW], BF16, name=f"corr_{s}",
                                    tag="corr")
            d['rr'] = [p_big.tile([128, HW], BF16, name=f"rr_{s}_{mt}",
                                  tag="big") for mt in range(2)]
            d['bnsr'] = [p_small.tile([128, NCH * 6], F32,
                                      name=f"bnsr_{s}_{mt}", tag="small")
                         for mt in range(2)]

        def phase2_chunk(s, ch):
            d = st[s]
            ipx = d['ipx']; khat = d['khat']
            istd1 = d['istd1']; bneg1 = d['bneg1']
            corr_sb = d['corr']; rr = d['rr']; bnsr = d['bnsr']
            if True:
                cp = [ps_ii.tile([S2, CHUNK], F32, name=f"c{i}_{s}_{ch}",
                                tag="ps_ii") for i in range(2)]
                for i, xd in ((0, x0_d), (1, x1_d)):
                    for t in range(2):
                        xc = p_stream.tile([128, CHUNK], BF16,
                                           name=f"xii{i}_{s}_{t}_{ch}",
                                           tag="stream")
                        nc.sync.dma_start(
                            xc[:], xd[s, t * 128:(t + 1) * 128,
                                      ch * CHUNK:(ch + 1) * CHUNK])
                        nc.tensor.matmul(cp[i][:],
                                         khat[i][t][:], xc[:],
                                         start=(t == 0), stop=(t == 1))
                        if i == 1:
                            # depth_feat chunk (reuses the x1 load)
                            dfc = p_df.tile([128, CHUNK], F32,
                                            name=f"dfc_{s}_{t}_{ch}", tag="df")
                            nc.vector.tensor_scalar(dfc[:],
                                                    xc[:],
                                                    istd1[t][:], bneg1[t][:],
                                                    ALU.mult, ALU.add)
                            nc.gpsimd.dma_start(
                                df_d[s, t * 128:(t + 1) * 128,
                                     ch * CHUNK:(ch + 1) * CHUNK], dfc[:])
                sl = slice(ch * CHUNK, (ch + 1) * CHUNK)
                tts = []
                for i in range(2):
                    t_ = p_tt.tile([S2, CHUNK], F32, name=f"tt{i}_{s}_{ch}",
                                   tag="tt")
                    nc.vector.tensor_tensor(t_[:], cp[i][:],
                                            ipx[ch][i * S2:(i + 1) * S2, :],
                                            ALU.mult)
                    tts.append(t_)
                nc.vector.tensor_tensor(corr_sb[:, sl], tts[0][:], tts[1][:],
                                        ALU.add)
                # red1 = w1 @ corr  (K = 64)
                for mt in range(2):
                    rrp = ps_ii.tile([128, CHUNK], F32,
                                    name=f"rrp_{s}_{ch}_{mt}", tag="ps_ii")
                    nc.tensor.matmul(rrp[:],
                                     w1t_sb[:, mt * 128:(mt + 1) * 128],
                                     corr_sb[:, sl], start=True, stop=True)
                    nc.scalar.copy(rr[mt][:, sl], rrp[:])
                    nc.vector.bn_stats(bnsr[mt][:, ch * 6:(ch + 1) * 6],
                                       rrp[:])

        def phase2_tail(s):
            d = st[s]
            rr = d['rr']; bnsr = d['bnsr']
            # IN(red) scale/bias, then relu into padded conv input
            red_pad = []
            for mt in range(2):
                mv = p_small.tile([128, 2], F32, name=f"mvr_{s}_{mt}",
                                  tag="small")
                nc.vector.bn_aggr(mv[:], bnsr[mt][:])
                std = p_small.tile([128, 1], F32, name=f"stdr_{s}_{mt}",
                                   tag="small")
                nc.scalar.activation(std[:], mv[:, 1:2], AF.Ln, bias=eps_c[:])
                ist = p_small.tile([128, 1], F32, name=f"istdr_{s}_{mt}",
                                   tag="small")
                nc.scalar.activation(ist[:], std[:], AF.Exp, scale=-0.5)
                bn = p_small.tile([128, 1], F32, name=f"bnegr_{s}_{mt}",
                                  tag="small")
                nc.vector.scalar_tensor_tensor(bn[:], mv[:, 0:1], -1.0, ist[:],
                                               ALU.mult, ALU.mult)
                pad = p_pad.tile([128, 66 * 66], BF16, name=f"pad_{s}_{mt}",
                                 tag="pad")
                pv = pad[:].rearrange("p (h w) -> p h w", w=66)
                for brd in (pv[:, 0:1, :], pv[:, 65:66, :],
                            pv[:, 1:65, 0:1], pv[:, 1:65, 65:66]):
                    nc.scalar.activation(brd, brd, AF.Copy,
                                         scale=0.0)
                nc.scalar.activation(
                    pv[:, 1:65, 1:65],
                    rr[mt][:].rearrange("p (h w) -> p h w", w=64),
                    AF.Relu, bias=bn[:], scale=ist[:])
                red_pad.append(pad)
            d['red_pad'] = red_pad

        def phase3(s, weave=None):
            d = st[s]
            red_pad = d['red_pad']
            # ===== phase III: conv3x3 ===== (r2 reuses the rr slots)
            r2 = [p_big.tile([128, HW], BF16, name=f"r2_{s}_{mt}", tag="big")
                  for mt in range(2)]
            d['r2'] = r2
            pvs = [red_pad[kt][:].rearrange("p (h w) -> p h w", w=66)
                   for kt in range(2)]
            for ch in range(NCH):               # 512-px chunks (8 rows)
                if weave is not None:
                    weave(ch)
                for mt in range(2):
                    c3p = ps_iii.tile([128, CHUNK], F32,
                                      name=f"c3_{s}_{mt}_{ch}", tag="ps_iii")
                    y0 = ch * 8
                    first = True
                    for off in range(9):
                        dy, dx = off // 3, off % 3
                        for kt in range(2):
                            lhs = w2t_sb[kt][:, off * C + mt * 128:
                                             off * C + mt * 128 + 128]
                            rhs = pvs[kt][:, y0 + dy:y0 + dy + 8, dx:dx + 64]
                            nc.tensor.matmul(
                                c3p[:], lhs, rhs,
                                start=first, stop=(off == 8 and kt == 1))
                            first = False
                    nc.scalar.activation(
                        r2[mt][:, ch * CHUNK:(ch + 1) * CHUNK],
                        c3p[:], AF.Identity, bias=b2_sb[:, mt:mt + 1])
                if weave is not None:
                    weave(NCH + ch)

        def phase4_unit(s, mt, g):
            d = st[s]
            r2 = d['r2']
            # ===== phase IV: adapt conv1x1 + relu (one 1024-px group) =====
            if True:
                if True:
                    gsl = slice(g * 1024, (g + 1) * 1024)
                    xas = []
                    for kt in range(2):
                        xa = p_xa.tile([128, 1024], BF16,
                                       name=f"xa_{s}_{mt}_{g}_{kt}", tag="xa")
                        nc.sync.dma_start(
                            xa[:], x0_d[s, kt * 128:(kt + 1) * 128, gsl])
                        xas.append(xa)
                    for cc in range(2):
                        ap_ = ps_ii.tile([128, CHUNK], F32,
                                        name=f"aps_{s}_{mt}_{g}_{cc}",
                                        tag="ps_ii")
                        csl = slice(cc * 512, (cc + 1) * 512)
                        for kt in range(4):
                            lhs = wat_sb[:, kt * C + mt * 128:
                                         kt * C + mt * 128 + 128]
                            if kt < 2:
                                rhs = xas[kt][:, csl]
                            else:
                                rhs = r2[kt - 2][:, g * 1024 + cc * 512:
                                                 g * 1024 + (cc + 1) * 512]
                            nc.tensor.matmul(ap_[:], lhs, rhs,
                                             start=(kt == 0), stop=(kt == 3))
                        nfc = p_nf.tile([128, CHUNK], F32,
                                        name=f"nf_{s}_{mt}_{g}_{cc}", tag="nf")
                        nc.scalar.activation(nfc[:], ap_[:], AF.Relu,
                                             bias=ba_sb[:, mt:mt + 1])
                        nc.gpsimd.dma_start(
                            nf_d[s, mt * 128:(mt + 1) * 128,
                                 g * 1024 + cc * 512:g * 1024 + (cc + 1) * 512],
                            nfc[:])


        phase1_head(0)
        for ch in range(NCH):
            phase1_chunk(0, ch)
        phase1_tail(0)
        phase2_head(0)
        for ch in range(NCH):
            phase2_chunk(0, ch)
        phase2_tail(0)
        phase1_head(1)

        # Weave slots fire in order 0,8,1,9,2,10,3,11,4,12,5,13,6,14,7,15
        # (slot k fires before conv chunk k; slot 8+k right after chunk k).
        # Pack sample 1's phase-I chunks into the first 8 fired slots so its
        # tail (and then phase II) unblocks by mid-window; adapt groups fill
        # the second half once their r2 chunks exist.
        _w0 = {0: 0, 8: 1, 1: 2, 9: 3, 2: 4, 10: 5, 3: 6, 11: 7}

        def _weave0(k):
            if k in _w0:
                phase1_chunk(1, _w0[k])
            elif k == 4:
                phase1_tail(1)
            elif k == 12:
                phase4_unit(0, 0, 0)
                phase4_unit(0, 1, 0)
            elif k == 5:
                phase4_unit(0, 0, 1)
                phase4_unit(0, 1, 1)
            elif k == 13:
                phase4_unit(0, 0, 2)
                phase4_unit(0, 1, 2)

        phase3(0, weave=_weave0)
        phase4_unit(0, 0, 3)
        phase4_unit(0, 1, 3)
        phase2_head(1)
        for ch in range(NCH):
            phase2_chunk(1, ch)
        phase2_tail(1)

        def _weave1(k):
            # weave sample 1's adapt groups into the conv3x3(1) window
            if k in (NCH + 1, NCH + 3, NCH + 5):
                g = (k - NCH - 1) // 2
                phase4_unit(1, 0, g)
                phase4_unit(1, 1, g)

        phase3(1, weave=_weave1)
        phase4_unit(1, 0, 3)
        phase4_unit(1, 1, 3)

    nc.compile()
    return nc


def _get_program():
    if "nc" not in _CACHE:
        _CACHE["nc"] = _build_program()
    return _CACHE["nc"]


def _prep_weights(w_red1, w_red2, w_adapt, b_red2, b_adapt):
    w1t = np.ascontiguousarray(w_red1[:, :, 0, 0].T)                  # [64,256]
    w2 = w_red2.transpose(2, 3, 1, 0).reshape(9, C, C)                # off,ci,co
    w2t = np.ascontiguousarray(
        w2.reshape(9, 2, 128, C).transpose(1, 2, 0, 3).reshape(2, 128, 9 * C))
    wat = np.ascontiguousarray(w_adapt[:, :, 0, 0].T.reshape(4, 128, C))
    b2 = np.ascontiguousarray(b_red2.reshape(2, 128))
    ba = np.ascontiguousarray(b_adapt.reshape(2, 128))
    return (w1t.astype(BF16_NP), w2t.astype(BF16_NP), wat.astype(BF16_NP),
            b2, ba)


def make_in_maps(x0, x1, w_red1, b_red1, w_red2, b_red2, w_adapt, b_adapt):
    w1t, w2t, wat, b2, ba = _prep_weights(
        np.asarray(w_red1, np.float32), np.asarray(w_red2, np.float32),
        np.asarray(w_adapt, np.float32), np.asarray(b_red2, np.float32),
        np.asarray(b_adapt, np.float32))
    x0 = np.asarray(x0, np.float32).reshape(B, C, HW).astype(BF16_NP)
    x1 = np.asarray(x1, np.float32).reshape(B, C, HW).astype(BF16_NP)
    in_maps = []
    for i in range(N_CORES):
        sl = slice(i * SPC, (i + 1) * SPC)
        in_maps.append({
            "x0": np.ascontiguousarray(x0[sl]),
            "x1": np.ascontiguousarray(x1[sl]),
            "w1t": w1t, "w2t": w2t, "wat": wat, "b2": b2, "ba": ba,
        })
    return in_maps


def kernel(x0, x1, w_red1, b_red1, w_red2, b_red2, w_adapt, b_adapt):
    nc = _get_program()
    in_maps = make_in_maps(x0, x1, w_red1, b_red1, w_red2, b_red2,
                           w_adapt, b_adapt)
    res = run_bass_kernel_spmd(nc, in_maps, list(range(N_CORES)))
    nf = np.concatenate([res.results[i]["nf"] for i in range(N_CORES)], axis=0)
    df = np.concatenate([res.results[i]["df"] for i in range(N_CORES)], axis=0)
    return (nf.reshape(B, C, H, W).astype(np.float32),
            df.reshape(B, C, H, W).astype(np.float32))
